# revision 25
# baseline (speedup 1.0000x reference)
"""HALE attention on 8 Trainium2 NeuronCores — bf16 rewrite.

Sharding: 2 heads/core (tensor-parallel), sequence-parallel tail after a
split AllToAll (core c owns rows [128c:128c+128] and [1024+128c:+128]).

Key layout choices vs the fp32 baseline:
  - all matmul operands bf16 (fp32 PSUM accumulation): single-pass MMs,
    FWL weight loads, half DMA bytes.
  - x^T staged on the host ([128, 8, 2048]); no on-device x transposes.
  - natural-orientation tensors built with XBAR dma_start_transpose on the
    SP/ACT DMA queues instead of PE transposes.
  - Haar block means + level projections batched into 512-col matmuls with
    the level dim stacked along free cols.
  - augmented-value tiles padded 65->68 cols so PSUM rows stay 8B-aligned.
  - AllToAll split in two (chunks 0-7 / 8-15); first half overlaps the
    second half of the recurrence.
"""

import numpy as np
import ml_dtypes
from contextlib import ExitStack

import concourse.bass as bass
import concourse.bacc as bacc
import concourse.tile as tile
import concourse.mybir as mybir
from concourse.bass_utils import run_bass_kernel_spmd

F32 = mybir.dt.float32
BF16 = mybir.dt.bfloat16
AF = mybir.ActivationFunctionType
OP = mybir.AluOpType
BF = ml_dtypes.bfloat16

NCORES = 8
N = 2048
DM = 1024
H = 16
DH = 64
L = 4
CH = 128
NCH = N // CH
WIN = 64
NSL = N // NCORES
EPS = 1e-6
VA = 68  # padded aug-value width (65 used, 68 for 8B PSUM row alignment)

_CACHE = {}
DEBUG = False


def _host_constants():
    ck = np.arange(CH)[:, None]
    cq = np.arange(CH)[None, :]
    maskT = (ck <= cq).astype(np.float32)
    prev = (ck >= cq + WIN + 1).astype(np.float32)
    cur = ((ck <= cq) & (ck >= cq - (WIN - 1))).astype(np.float32)
    lmask = np.concatenate([prev, cur], axis=1)
    Ml = np.zeros((L, CH, CH), dtype=np.float32)
    for lv in range(L):
        b = 2 ** (lv + 1)
        m = np.arange(CH)[:, None]
        n = np.arange(CH)[None, :]
        Ml[lv] = np.where(((m // b) == (n // b)) & (m <= n),
                          1.0 / (n % b + 1.0), 0.0)
    maskT4 = np.tile(maskT, (1, 4))
    Ml_all = Ml.transpose(1, 0, 2).reshape(CH, L * CH)
    return maskT4, lmask, Ml_all


def _blockdiag2(a):
    z = np.zeros((128, 128), dtype=np.float32)
    z[:64, :64] = a
    z[64:, 64:] = a
    return z


def _build_nc():
    nc = bacc.Bacc("TRN2", target_bir_lowering=False, debug=False,
                   num_devices=NCORES)

    xT_d = nc.dram_tensor("xT", [128, 8 * N], BF16, kind="ExternalInput")
    wproj_d = nc.dram_tensor("wproj", [128, 5 * 8 * 128], BF16,
                             kind="ExternalInput")
    bdWkT_d = nc.dram_tensor("bdWkT", [128, L * 128], BF16,
                             kind="ExternalInput")
    bdWvT_d = nc.dram_tensor("bdWvT", [128, L * 128], BF16,
                             kind="ExternalInput")
    Ml_d = nc.dram_tensor("Ml", [128, L * 128], BF16, kind="ExternalInput")
    maskT4_d = nc.dram_tensor("maskT4", [128, 512], BF16,
                              kind="ExternalInput")
    lmask_d = nc.dram_tensor("lmask", [128, 256], BF16, kind="ExternalInput")
    ident_d = nc.dram_tensor("ident", [128, 128], BF16, kind="ExternalInput")
    w5b_d = nc.dram_tensor("w5b", [128, 5], F32, kind="ExternalInput")
    wgT_d = nc.dram_tensor("wgT", [2 * DM, DM], BF16, kind="ExternalInput")
    woT_d = nc.dram_tensor("woT", [DM, DM], BF16, kind="ExternalInput")
    wgo8_d = nc.dram_tensor("wgo8", [128, 8], BF16, kind="ExternalInput")
    bg_d = nc.dram_tensor("bg", [1, DM], BF16, kind="ExternalInput")
    bo_d = nc.dram_tensor("bo", [1, DM], BF16, kind="ExternalInput")
    bgo_d = nc.dram_tensor("bgo", [128, 1], F32, kind="ExternalInput")
    out_d = nc.dram_tensor("out", [2 * 128, DM], F32, kind="ExternalOutput")

    dbg_d = {}
    if DEBUG:
        for nm, w in (("qT", N), ("kpT", N), ("knat", N), ("kpnat", N),
                      ("vnat", N), ("vlnat", N), ("kplT", L * N),
                      ("kplN", L * N), ("vlvA", L * NCH * 2 * VA),
                      ("vaug", NCH * 2 * VA), ("glob", N), ("loc", N),
                      ("S", 5 * VA), ("atm", 640)):
            dbg_d[nm] = nc.dram_tensor(f"dbg_{nm}", [128, w], BF16,
                                       kind="ExternalOutput")
        for nm, w in (("pso", 5 * VA), ("dmax", 5), ("rw", 5)):
            dbg_d[nm] = nc.dram_tensor(f"dbg_{nm}", [128, w], F32,
                                       kind="ExternalOutput")

    # [dest core, tensor(diff,glob), 128, 128] for chunk halves A/B
    a2aA_in = nc.dram_tensor("a2aA_in", [NCORES, 2, 128, 128], BF16)
    a2aA_out = nc.dram_tensor("a2aA_out", [NCORES, 2, 128, 128], BF16)
    a2aB_in = nc.dram_tensor("a2aB_in", [NCORES, 2, 128, 128], BF16)
    a2aB_out = nc.dram_tensor("a2aB_out", [NCORES, 2, 128, 128], BF16)

    with tile.TileContext(nc) as tc, ExitStack() as root:
        cpool = root.enter_context(tc.tile_pool(name="consts", bufs=1))
        persist = root.enter_context(tc.tile_pool(name="persist", bufs=1))

        maskT4 = cpool.tile([128, 512], BF16)
        lmask = cpool.tile([128, 256], BF16)
        Ml_sb = cpool.tile([128, L * 128], BF16)
        bdWkT = cpool.tile([128, L * 128], BF16)
        bdWvT = cpool.tile([128, L * 128], BF16)
        ident = cpool.tile([128, 128], BF16)
        w5b = cpool.tile([128, 5], F32)
        ones_row = cpool.tile([1, 128], BF16)
        bg_sb = cpool.tile([1, DM], BF16)
        bo_sb = cpool.tile([1, DM], BF16)
        bgo_sb = cpool.tile([128, 1], F32)
        wgo_sb = cpool.tile([128, 8], BF16)
        nc.scalar.dma_start(maskT4[:], maskT4_d[:])
        nc.scalar.dma_start(lmask[:], lmask_d[:])
        nc.sync.dma_start(ident[:], ident_d[:])
        nc.sync.dma_start(Ml_sb[:], Ml_d[:])
        nc.gpsimd.dma_start(bdWkT[:], bdWkT_d[:])
        nc.gpsimd.dma_start(bdWvT[:], bdWvT_d[:])
        nc.scalar.dma_start(w5b[:], w5b_d[:])
        nc.scalar.dma_start(bg_sb[:], bg_d[:])
        nc.scalar.dma_start(bo_sb[:], bo_d[:])
        nc.scalar.dma_start(bgo_sb[:], bgo_d[:])
        nc.scalar.dma_start(wgo_sb[:], wgo8_d[:])
        nc.vector.memset(ones_row[:], 1.0)

        glob = persist.tile([128, N], BF16)
        loc = persist.tile([128, N], BF16)
        S_bf = persist.tile([128, 5, VA], BF16)
        xT = persist.tile([128, 8, N], BF16)
        qT = persist.tile([128, N], BF16)
        klT = persist.tile([128, N], BF16)
        qpT = persist.tile([128, N], BF16)
        kpT = persist.tile([128, N], BF16)
        knat = persist.tile([128, N], BF16)
        kpnat = persist.tile([128, N], BF16)
        vnat = persist.tile([128, N], BF16)
        vlnat = persist.tile([128, N], BF16)
        kplT = persist.tile([128, L, N], BF16)
        kplN = persist.tile([128, L, N], BF16)
        vaug = persist.tile([128, NCH, 2, VA], BF16)
        vlaug = persist.tile([128, NCH, 2, VA], BF16)
        vlvA = persist.tile([128, L, NCH, 2, VA], BF16)
        if DEBUG:
            dpso = persist.tile([128, 5, VA], F32)
            datm = persist.tile([128, 640], BF16)
            ddmax = persist.tile([128, 5], F32)
            drw = persist.tile([128, 5], F32)

        nc.vector.memset(vaug[:, :, :, 64:65], 1.0)
        nc.vector.memset(vlaug[:, :, :, 64:65], 1.0)
        nc.vector.memset(vlvA[:, :, :, :, 64:65], 1.0)

        def cp(dst, src, on_scalar=False):
            if on_scalar:
                nc.scalar.activation(dst, src, AF.Copy)
            else:
                nc.vector.tensor_copy(dst, src)

        with ExitStack() as phA:
            tmp_p = phA.enter_context(tc.tile_pool(name="phitmp", bufs=2))
            phPrj = phA.enter_context(ExitStack())
            prj = phPrj.enter_context(tc.tile_pool(name="prj", bufs=1))
            phP = phA.enter_context(ExitStack())
            ps_pr = phP.enter_context(
                tc.tile_pool(name="ps_pr", bufs=2, space="PSUM"))

            wproj = prj.tile([128, 5 * 8 * 128], BF16)
            nc.sync.dma_start(wproj[:], wproj_d[:])
            kT = prj.tile([128, N], BF16)
            vT = prj.tile([128, N], BF16)
            vlT = prj.tile([128, N], BF16)

            def phi_big(dst, src):
                tmp = tmp_p.tile([128, N], BF16, tag="phitmp")
                nc.vector.tensor_scalar_min(tmp[:], src[:], 0.0)
                nc.scalar.activation(dst[:], tmp[:], AF.Exp)
                nc.vector.scalar_tensor_tensor(
                    dst[:], src[:], 0.0, dst[:], op0=OP.max, op1=OP.add)

            # ----- projections (k-outer, 4x512-col accumulators) -----
            for ip, dstT in enumerate((qT, kT, klT, vT, vlT)):
                accs = [ps_pr.tile([128, 512], F32, tag=f"pa{nb}",
                                   name=f"acc{nb}")
                        for nb in range(4)]
                for k in range(8):
                    if ip == 0:
                        nc.gpsimd.dma_start(xT[:, k, :],
                                            xT_d[:, N * k:N * (k + 1)])
                    for nb in range(4):
                        nc.tensor.matmul(
                            accs[nb][:], wproj[:, (ip * 8 + k) * 128:
                                                (ip * 8 + k + 1) * 128],
                            xT[:, k, 512 * nb:512 * (nb + 1)],
                            start=(k == 0), stop=(k == 7))
                for nb in range(4):
                    cp(dstT[:, 512 * nb:512 * (nb + 1)], accs[nb][:],
                       on_scalar=(nb % 2 == 1))
                if ip == 0:
                    phi_big(qpT, qT)
                elif ip == 1:
                    phi_big(kpT, kT)
                elif ip == 3:
                    for i in range(NCH):
                        sl = slice(CH * i, CH * (i + 1))
                        nc.sync.dma_start_transpose(vnat[:, sl], vT[:, sl])
                        for h in range(2):
                            nc.gpsimd.tensor_copy(
                                vaug[:, i, h, 0:64],
                                vnat[:, CH * i + 64 * h:CH * i + 64 * h + 64])
                elif ip == 4:
                    for i in range(NCH):
                        sl = slice(CH * i, CH * (i + 1))
                        nc.scalar.dma_start_transpose(vlnat[:, sl], vlT[:, sl])
                        for h in range(2):
                            nc.gpsimd.tensor_copy(
                                vlaug[:, i, h, 0:64],
                                vlnat[:, CH * i + 64 * h:CH * i + 64 * h + 64])

            # ----- Haar: block means + level projections (batched) -----
            phP.close()
            bm_p = phA.enter_context(tc.tile_pool(name="bm", bufs=2))
            ps_bm = phA.enter_context(
                tc.tile_pool(name="ps_bm", bufs=2, space="PSUM"))
            ps_trH = phA.enter_context(
                tc.tile_pool(name="ps_trH", bufs=2, space="PSUM"))
            for i in range(NCH):
                sl = slice(CH * i, CH * (i + 1))
                ptn = ps_trH.tile([128, 128], BF16, tag="ptk", bufs=1)
                nc.tensor.transpose(ptn[:], kT[:, sl], ident[:])
                cp(knat[:, sl], ptn[:], on_scalar=(i % 2 == 1))
            phi_big(kpnat, knat)
            for g in range(4):
                bmk = bm_p.tile([128, L, 512], BF16, tag="bmk", bufs=2)
                bmv = bm_p.tile([128, L, 512], BF16, tag="bmv", bufs=2)
                for ig in range(4):
                    i = 4 * g + ig
                    sl = slice(CH * i, CH * (i + 1))
                    psk = ps_bm.tile([128, 512], F32, tag="bmk", bufs=1)
                    nc.tensor.matmul(psk[:], knat[:, sl], Ml_sb[:],
                                     start=True, stop=True)
                    nc.vector.tensor_copy(
                        bmk[:].rearrange("p l (c s) -> p l c s",
                                         c=4)[:, :, ig, :],
                        psk[:].rearrange("p (l s) -> p l s", l=L))
                    psv = ps_bm.tile([128, 512], F32, tag="bmv", bufs=1)
                    nc.tensor.matmul(psv[:], vnat[:, sl], Ml_sb[:],
                                     start=True, stop=True)
                    nc.scalar.activation(
                        bmv[:].rearrange("p l (c s) -> p l c s",
                                         c=4)[:, :, ig, :],
                        psv[:].rearrange("p (l s) -> p l s", l=L), AF.Copy)
                vlvT = bm_p.tile([128, L, 512], BF16, tag="vlvT")
                nsl = slice(512 * g, 512 * (g + 1))
                for lv in range(L):
                    pst = ps_bm.tile([128, 512], F32, tag="kt", bufs=2)
                    nc.tensor.matmul(pst[:],
                                     bdWkT[:, 128 * lv:128 * (lv + 1)],
                                     bmk[:, lv, :], start=True, stop=True)
                    tmp = tmp_p.tile([128, 512], BF16, tag="phs")
                    nc.vector.tensor_scalar_min(tmp[:], pst[:], 0.0)
                    nc.scalar.activation(kplT[:, lv, nsl], tmp[:], AF.Exp)
                    nc.vector.scalar_tensor_tensor(
                        kplT[:, lv, nsl], pst[:], 0.0, kplT[:, lv, nsl],
                        op0=OP.max, op1=OP.add)
                    psvt = ps_bm.tile([128, 512], F32, tag="vt", bufs=2)
                    nc.tensor.matmul(psvt[:],
                                     bdWvT[:, 128 * lv:128 * (lv + 1)],
                                     bmv[:, lv, :], start=True, stop=True)
                    nc.vector.tensor_copy(vlvT[:, lv, :], psvt[:])
                for lv in range(L):
                    for ig in range(4):
                        i = 4 * g + ig
                        sl = slice(CH * i, CH * (i + 1))
                        gsl = slice(CH * ig, CH * (ig + 1))
                        ptk = ps_trH.tile([128, 128], BF16, tag="ptk",
                                          bufs=1)
                        nc.tensor.transpose(ptk[:], kplT[:, lv, sl], ident[:])
                        cp(kplN[:, lv, sl], ptk[:],
                           on_scalar=(ig % 2 == 1))
                        ptv = ps_trH.tile([128, 128], BF16, tag="ptv",
                                          bufs=1)
                        nc.tensor.transpose(ptv[:], vlvT[:, lv, gsl], ident[:])
                        for h in range(2):
                            cp(vlvA[:, lv, i, h, 0:64],
                               ptv[:, 64 * h:64 * h + 64],
                               on_scalar=(h == 1))

        # ----- chunk-major recurrence + local attention -----
        with ExitStack() as phB:
            atm_p = phB.enter_context(tc.tile_pool(name="atm", bufs=3))
            tin_p = phB.enter_context(tc.tile_pool(name="tiny", bufs=4))
            ps_Aa = phB.enter_context(
                tc.tile_pool(name="ps_Aa", bufs=2, space="PSUM"))
            ps_Ab = phB.enter_context(
                tc.tile_pool(name="ps_Ab", bufs=1, space="PSUM"))
            ps_O = phB.enter_context(
                tc.tile_pool(name="ps_O", bufs=2, space="PSUM"))
            ps_Sd = phB.enter_context(
                tc.tile_pool(name="ps_Sd", bufs=1, space="PSUM"))

            for i in range(NCH):
                sl = slice(CH * i, CH * (i + 1))
                psSd = ps_Sd.tile([128, 7, VA], F32, tag="psSd")
                atms, Ps, psLs = [], [], []
                for h in range(2):
                    hp = slice(64 * h, 64 * h + 64)
                    psa_a = ps_Aa.tile([128, 512], F32, tag="psa")
                    psab = ps_Ab.tile([128, 384], F32, tag="psb", bufs=2)
                    for lv in range(5):
                        lhsT = (kpT[hp, sl] if lv == 0
                                else kplT[hp, lv - 1, sl])
                        out = (psa_a[:, 128 * (lv - 1):128 * lv] if lv > 0
                               else psab[:, 0:128])
                        nc.tensor.matmul(out, lhsT, qpT[hp, sl],
                                         start=True, stop=True)
                    if i > 0:
                        nc.tensor.matmul(psab[:, 128:256],
                                         klT[hp, CH * (i - 1):CH * i],
                                         qT[hp, sl], start=True, stop=True)
                    nc.tensor.matmul(psab[:, 256:384], klT[hp, sl],
                                     qT[hp, sl], start=True, stop=True)
                    atm = atm_p.tile([128, 640], BF16, tag="atm")
                    nc.vector.tensor_mul(atm[:, 0:512], psa_a[:], maskT4[:])
                    nc.vector.tensor_mul(atm[:, 512:640], psab[:, 0:128],
                                         maskT4[:, 0:128])
                    P = atm_p.tile([128, 256], BF16, tag="P")
                    if i > 0:
                        nc.scalar.activation(P[:], psab[:, 128:384], AF.Exp,
                                             scale=0.125)
                        nc.vector.tensor_mul(P[:], P[:], lmask[:])
                    else:
                        nc.scalar.activation(P[:, 128:256], psab[:, 256:384],
                                             AF.Exp, scale=0.125)
                        nc.vector.tensor_mul(P[:, 128:256], P[:, 128:256],
                                             lmask[:, 128:256])
                    atms.append(atm)
                    Ps.append(P)
                for h in range(2):
                    hp = slice(64 * h, 64 * h + 64)
                    c0 = CH * i + 64 * h
                    atm, P = atms[h], Ps[h]
                    pso = ps_O.tile([128, 5, VA], F32, tag="pso")
                    for lv in range(5):
                        alv = atm[:, 512:640] if lv == 0 else \
                            atm[:, 128 * (lv - 1):128 * lv]
                        va_l = (vaug[:, i, h, 0:65] if lv == 0
                                else vlvA[:, lv - 1, i, h, 0:65])
                        nc.tensor.matmul(pso[:, lv, 0:65], alv, va_l,
                                         start=True, stop=(i == 0))
                        if i > 0:
                            nc.tensor.matmul(pso[:, lv, 0:65], qpT[hp, sl],
                                             S_bf[hp, lv, 0:65],
                                             start=False, stop=True)
                    for lv in range(5):
                        kn_l = (kpnat[:, c0:c0 + 64] if lv == 0
                                else kplN[:, lv - 1, c0:c0 + 64])
                        va_l = (vaug[:, i, h, 0:65] if lv == 0
                                else vlvA[:, lv - 1, i, h, 0:65])
                        nc.tensor.matmul(psSd[hp, lv, 0:65], kn_l, va_l,
                                         start=True, stop=True)
                    if i > 0:
                        nc.tensor.matmul(psSd[:, 5 + h, 0:65], P[:, 0:128],
                                         vlaug[:, i - 1, h, 0:65],
                                         start=True, stop=False)
                    nc.tensor.matmul(psSd[:, 5 + h, 0:65], P[:, 128:256],
                                     vlaug[:, i, h, 0:65],
                                     start=(i == 0), stop=True)
                    dmax = tin_p.tile([128, 5], F32, tag="dmax")
                    if DEBUG and i == 1 and h == 0:
                        nc.vector.tensor_copy(dpso[:], pso[:])
                        nc.vector.tensor_copy(datm[:], atm[:])
                    nc.vector.tensor_scalar_max(dmax[:], pso[:, :, 64], EPS)
                    rec = tin_p.tile([128, 5], F32, tag="rec")
                    nc.vector.reciprocal(rec[:], dmax[:])
                    rw = tin_p.tile([128, 5], F32, tag="rw")
                    nc.vector.tensor_mul(rw[:], rec[:], w5b[:])
                    if DEBUG and i == 1 and h == 0:
                        nc.vector.tensor_copy(ddmax[:], dmax[:])
                        nc.vector.tensor_copy(drw[:], rw[:])
                    gsl = glob[:, c0:c0 + 64]
                    nc.vector.tensor_scalar_mul(gsl, pso[:, 0, 0:64],
                                                rw[:, 0:1])
                    for lv in range(1, 5):
                        nc.vector.scalar_tensor_tensor(
                            gsl, pso[:, lv, 0:64], rw[:, lv:lv + 1], gsl,
                            op0=OP.mult, op1=OP.add)
                    dm = tin_p.tile([128, 1], F32, tag="dm")
                    nc.vector.tensor_scalar_max(dm[:], psSd[:, 5 + h, 64:65],
                                                1e-30)
                    rl = tin_p.tile([128, 1], F32, tag="rl")
                    nc.vector.reciprocal(rl[:], dm[:])
                    nc.scalar.mul(loc[:, c0:c0 + 64], psSd[:, 5 + h, 0:64],
                                  rl[:])
                if i == 0:
                    nc.vector.tensor_copy(S_bf[:, :, 0:65],
                                          psSd[:, 0:5, 0:65])
                else:
                    nc.vector.tensor_add(S_bf[:, :, 0:65], S_bf[:, :, 0:65],
                                         psSd[:, 0:5, 0:65])
                # diff in place + stage this chunk for its dest core
                nc.vector.tensor_sub(loc[:, sl], loc[:, sl], glob[:, sl])
                dst = a2aA_in if i < 8 else a2aB_in
                nc.gpsimd.dma_start(dst.ap()[i % 8, 0], loc[:, sl])
                nc.gpsimd.dma_start(dst.ap()[i % 8, 1], glob[:, sl])
                if i == 7:
                    nc.gpsimd.collective_compute(
                        "AllToAll", OP.bypass,
                        ins=[a2aA_in.ap().opt()], outs=[a2aA_out.ap().opt()],
                        replica_groups=[list(range(NCORES))])
            nc.gpsimd.collective_compute(
                "AllToAll", OP.bypass,
                ins=[a2aB_in.ap().opt()], outs=[a2aB_out.ap().opt()],
                replica_groups=[list(range(NCORES))])

        # ---------- sequence-parallel tail ----------
        with ExitStack() as phC:
            tl = phC.enter_context(tc.tile_pool(name="tail", bufs=1))
            wst = phC.enter_context(tc.tile_pool(name="wstream", bufs=6))
            ps_tr2 = phC.enter_context(
                tc.tile_pool(name="ps_tr2", bufs=2, space="PSUM"))
            ps_g = phC.enter_context(
                tc.tile_pool(name="ps_g", bufs=1, space="PSUM"))

            diff_g = tl.tile([128, 2, DM], BF16)
            glob_g = tl.tile([128, 2, DM], BF16)
            for t2, (src_o) in enumerate((a2aA_out, a2aB_out)):
                nc.sync.dma_start(
                    diff_g[:, t2, :].rearrange("p (s m) -> p s m", s=8),
                    src_o.ap()[:, 0].rearrange("s p m -> p s m"))
                nc.sync.dma_start(
                    glob_g[:, t2, :].rearrange("p (s m) -> p s m", s=8),
                    src_o.ap()[:, 1].rearrange("s p m -> p s m"))

            pid = nc.sync.partition_id()
            r0 = pid * 128
            # x^T columns for this core's rows, staged to a static tile
            xTv = xT[:].rearrange("p k (t n) -> p k t n", t=2)
            xslT = tl.tile([128, 8, 2, 128], BF16)
            nc.sync.dma_start(xslT[:], xTv[:, :, :, bass.ds(r0, 128)])

            psG = [ps_g.tile([128, 512], F32, tag=f"psG{j}", name=f"psG{j}")
                   for j in range(4)]
            for kc in range(8):
                wg_t = wst.tile([128, DM], BF16, tag="wg")
                nc.sync.dma_start(wg_t[:], wgT_d[128 * kc:128 * (kc + 1), :])
                for t2 in range(2):
                    lhs = xslT[:, kc, t2, :]
                    for g2 in range(2):
                        nc.tensor.matmul(
                            psG[2 * t2 + g2][:], lhs,
                            wg_t[:, 512 * g2:512 * (g2 + 1)],
                            start=(kc == 0), stop=False)
            diffT = tl.tile([128, 8, 256], BF16)
            for t2 in range(2):
                for k in range(8):
                    pt = ps_tr2.tile([128, 128], BF16, tag="ptr2")
                    nc.tensor.transpose(
                        pt[:], diff_g[:, t2, 128 * k:128 * (k + 1)], ident[:])
                    cp(diffT[:, k, 128 * t2:128 * (t2 + 1)], pt[:],
                       on_scalar=(k % 2 == 1))
            for kc in range(8, 16):
                wg_t = wst.tile([128, DM], BF16, tag="wg")
                nc.sync.dma_start(wg_t[:], wgT_d[128 * kc:128 * (kc + 1), :])
                for t2 in range(2):
                    lhs = diffT[:, kc - 8, 128 * t2:128 * (t2 + 1)]
                    for g2 in range(2):
                        nc.tensor.matmul(
                            psG[2 * t2 + g2][:], lhs,
                            wg_t[:, 512 * g2:512 * (g2 + 1)],
                            start=False, stop=False)
            gh = tl.tile([128, 2, DM], BF16)
            for t2 in range(2):
                for g2 in range(2):
                    nc.tensor.matmul(
                        psG[2 * t2 + g2][:], ones_row[:],
                        bg_sb[:, 512 * g2:512 * (g2 + 1)],
                        start=False, stop=True)
                    nc.scalar.activation(
                        gh[:, t2, 512 * g2:512 * (g2 + 1)],
                        psG[2 * t2 + g2][:], AF.Silu)

            ghT = tl.tile([128, 8, 256], BF16)
            for t2 in range(2):
                for k in range(8):
                    pt = ps_tr2.tile([128, 128], BF16, tag="ptr2")
                    nc.tensor.transpose(
                        pt[:], gh[:, t2, 128 * k:128 * (k + 1)], ident[:])
                    cp(ghT[:, k, 128 * t2:128 * (t2 + 1)], pt[:],
                       on_scalar=(k % 2 == 1))

            psAl = ps_tr2.tile([128, 2], F32, tag="psAl")
            for t2 in range(2):
                for gc in range(8):
                    nc.tensor.matmul(psAl[:, t2:t2 + 1],
                                     ghT[:, gc, 128 * t2:128 * (t2 + 1)],
                                     wgo_sb[:, gc:gc + 1],
                                     start=(gc == 0), stop=(gc == 7))
            alpha = tl.tile([128, 2], F32)
            nc.scalar.activation(alpha[:], psAl[:], AF.Sigmoid, bias=bgo_sb[:])

            mx = tl.tile([128, 2, DM], BF16)
            for t2 in range(2):
                nc.vector.scalar_tensor_tensor(
                    mx[:, t2, :], diff_g[:, t2, :], alpha[:, t2:t2 + 1],
                    glob_g[:, t2, :], op0=OP.mult, op1=OP.add)
            mxT = tl.tile([128, 8, 256], BF16)
            for t2 in range(2):
                for k in range(8):
                    pt = ps_tr2.tile([128, 128], BF16, tag="ptr2")
                    nc.tensor.transpose(
                        pt[:], mx[:, t2, 128 * k:128 * (k + 1)], ident[:])
                    cp(mxT[:, k, 128 * t2:128 * (t2 + 1)], pt[:],
                       on_scalar=(k % 2 == 1))

            out_sb = tl.tile([128, 2, DM], F32)
            psF = [ps_g.tile([128, 512], F32, tag=f"psG{j}", name=f"psF{j}")
                   for j in range(4)]
            for kc in range(8):
                wo_t = wst.tile([128, DM], BF16, tag="wo")
                nc.sync.dma_start(wo_t[:], woT_d[128 * kc:128 * (kc + 1), :])
                for t2 in range(2):
                    for o2 in range(2):
                        nc.tensor.matmul(
                            psF[2 * t2 + o2][:],
                            mxT[:, kc, 128 * t2:128 * (t2 + 1)],
                            wo_t[:, 512 * o2:512 * (o2 + 1)],
                            start=(kc == 0), stop=False)
            for t2 in range(2):
                for o2 in range(2):
                    nc.tensor.matmul(
                        psF[2 * t2 + o2][:], ones_row[:],
                        bo_sb[:, 512 * o2:512 * (o2 + 1)],
                        start=False, stop=True)
                    cp(out_sb[:, t2, 512 * o2:512 * (o2 + 1)],
                       psF[2 * t2 + o2][:], on_scalar=(o2 == 1))

            nc.sync.dma_start(
                out_d.ap().rearrange("(a b) c -> b a c", b=128), out_sb[:])

        if DEBUG:
            for nm, t in (("qT", qT), ("kpT", kpT), ("knat", knat),
                          ("kpnat", kpnat), ("vnat", vnat), ("vlnat", vlnat),
                          ("glob", glob), ("loc", loc)):
                nc.gpsimd.dma_start(dbg_d[nm].ap(), t[:])
            nc.gpsimd.dma_start(
                dbg_d["kplT"].ap().rearrange("p (l n) -> p l n", l=L), kplT[:])
            nc.gpsimd.dma_start(
                dbg_d["kplN"].ap().rearrange("p (l n) -> p l n", l=L), kplN[:])
            nc.gpsimd.dma_start(
                dbg_d["vlvA"].ap().rearrange(
                    "p (l c h v) -> p l c h v", l=L, c=NCH, h=2), vlvA[:])
            nc.gpsimd.dma_start(
                dbg_d["vaug"].ap().rearrange(
                    "p (c h v) -> p c h v", c=NCH, h=2), vaug[:])
            nc.gpsimd.dma_start(
                dbg_d["S"].ap().rearrange("p (l v) -> p l v", l=5), S_bf[:])
            nc.gpsimd.dma_start(
                dbg_d["pso"].ap().rearrange("p (l v) -> p l v", l=5), dpso[:])
            nc.gpsimd.dma_start(dbg_d["atm"].ap(), datm[:])
            nc.gpsimd.dma_start(dbg_d["dmax"].ap(), ddmax[:])
            nc.gpsimd.dma_start(dbg_d["rw"].ap(), drw[:])

    nc.compile()
    return nc


def _prep_in_maps(x, Wq, Wk, Wv, Wkl, Wvl, haar_Wk, haar_Wv, haar_scale,
                  Wg, bg, Wgo, bgo, Wo, bo):
    maskT4, lmask, Ml_all = _host_constants()
    x2 = np.asarray(x, dtype=np.float32).reshape(N, DM)
    xT = np.ascontiguousarray(
        x2.reshape(N, 8, 128).transpose(2, 1, 0).reshape(128, 8 * N)
    ).astype(BF)
    bdWkT = np.concatenate(
        [_blockdiag2(np.asarray(haar_Wk[lv], dtype=np.float32).T)
         for lv in range(L)], axis=1)
    bdWvT = np.concatenate(
        [_blockdiag2(np.asarray(haar_Wv[lv], dtype=np.float32).T)
         for lv in range(L)], axis=1)
    hs = np.asarray(haar_scale, dtype=np.float64)
    sw = np.exp(hs - hs.max())
    sw = (sw / sw.sum()).astype(np.float32)
    w5b = np.tile(np.concatenate([[1.0], sw]).astype(np.float32)[None, :],
                  (128, 1))
    shared = {
        "xT": xT,
        "bdWkT": bdWkT.astype(BF), "bdWvT": bdWvT.astype(BF),
        "Ml": Ml_all.astype(BF), "maskT4": maskT4.astype(BF),
        "lmask": lmask.astype(BF),
        "ident": np.eye(128, dtype=np.float32).astype(BF),
        "w5b": w5b,
        "wgT": np.ascontiguousarray(
            np.asarray(Wg, dtype=np.float32).T).astype(BF),
        "woT": np.ascontiguousarray(
            np.asarray(Wo, dtype=np.float32).T).astype(BF),
        "wgo8": np.ascontiguousarray(
            np.asarray(Wgo, dtype=np.float32).reshape(8, 128).T).astype(BF),
        "bg": np.asarray(bg, dtype=np.float32).reshape(1, DM).astype(BF),
        "bo": np.asarray(bo, dtype=np.float32).reshape(1, DM).astype(BF),
        "bgo": np.full((128, 1), np.asarray(bgo, dtype=np.float32).reshape(()),
                       dtype=np.float32),
    }
    in_maps = []
    for c in range(NCORES):
        sc = slice(128 * c, 128 * (c + 1))
        m = dict(shared)
        wp = np.empty((128, 5, 8, 128), dtype=np.float32)
        for ip, W in enumerate((Wq, Wk, Wkl, Wv, Wvl)):
            Wc = np.asarray(W, dtype=np.float32)[sc, :]  # [128 m, 1024 dm]
            wp[:, ip] = Wc.reshape(128, 8, 128).transpose(2, 1, 0)
        m["wproj"] = np.ascontiguousarray(
            wp.reshape(128, 5 * 8 * 128)).astype(BF)
        in_maps.append(m)
    return in_maps


def kernel_run(inputs, trace=False):
    if "nc" not in _CACHE:
        _CACHE["nc"] = _build_nc()
    nc = _CACHE["nc"]
    in_maps = _prep_in_maps(**inputs)
    res = run_bass_kernel_spmd(nc, in_maps, list(range(NCORES)), trace=trace)
    out = np.empty((N, DM), dtype=np.float32)
    for c in range(NCORES):
        out[128 * c:128 * (c + 1)] = res.results[c]["out"][0:128]
        out[1024 + 128 * c:1024 + 128 * (c + 1)] = res.results[c]["out"][128:256]
    return out.reshape(1, N, DM), res


def kernel(**inputs):
    out, _ = kernel_run(inputs, trace=False)
    return out
